# revision 17
# baseline (speedup 1.0000x reference)
"""DistanceAwareMultiheadAttention Trainium2 kernel.

Sharding: 8 cores = 4 batches x 2 head-groups (4 heads each).
Per-core layout ("S^T layout"): attention matrices are stored as E^T[r, q]
(key index r on partitions, query index q on free dim) so the A@V matmul
consumes E^T directly as the moving operand with V as stationary weights
(lhsT), producing O^T = [d, q] plus fused row-sums via a ones-column.

Math identities used (exact):
  - softmax is invariant to adding f(q) to logits => the a1[q]-only bias
    term cancels; only dist*(u[q]+w[r]) and c1[r] remain.
      u = q@(Ek0-Ek1)/8, a1 = q@Ek1/8 (cancelled)
      w = k@(Eq0-Eq1)/8, c1 = k@Eq1/8
  - 2*q@k / 8 is folded into the Q projection (scale 0.25).
  - dist^2 is a rank-4 bilinear form => computed by one K=4 matmul,
    dist = sqrt(.) on ACT. dist is shared by all 4 heads on the core.
  - out = Av/s + (t0/s)*(Ev0-Ev1) + Ev1, with s = sum_r E (ones column in
    the AV matmul) and t0 = sum_r E*dist (ones-lhsT matmul over E*dist).
"""

import numpy as np

import concourse.bass as bass
import concourse.mybir as mybir
import concourse.tile as tile
from concourse import bacc
from concourse.masks import make_identity

P = 128
S = 1024
E = 512
H_PER_CORE = 4
EC = E // P          # 4 e-chunks
R = S // P           # 8 r-chunks
DH = 64              # head dim
NH = H_PER_CORE
MAX_DIST = 100000.0 * np.sqrt(2.0)
FP = mybir.dt.float32

_BUILD_CACHE = {}


def build_bass():
    if "nc" in _BUILD_CACHE:
        return _BUILD_CACHE["nc"]
    nc = bacc.Bacc("TRN2", debug=False)

    # DRAM I/O (per-core shard tensors; same NEFF on all 8 cores)
    qT = nc.dram_tensor("qT", [E, S], FP, kind="ExternalInput").ap()
    kT = nc.dram_tensor("kT", [E, S], FP, kind="ExternalInput").ap()
    vT = nc.dram_tensor("vT", [E, S], FP, kind="ExternalInput").ap()
    wq = nc.dram_tensor("wq", [E, 256], FP, kind="ExternalInput").ap()
    wk = nc.dram_tensor("wk", [E, 256], FP, kind="ExternalInput").ap()
    wv = nc.dram_tensor("wv", [E, 256], FP, kind="ExternalInput").ap()
    wu = nc.dram_tensor("wu", [E, NH], FP, kind="ExternalInput").ap()
    wwc = nc.dram_tensor("wwc", [E, 2 * NH], FP, kind="ExternalInput").ap()
    bq = nc.dram_tensor("bq", [1, 256], FP, kind="ExternalInput").ap()
    bk = nc.dram_tensor("bk", [1, 256], FP, kind="ExternalInput").ap()
    bv = nc.dram_tensor("bv", [1, 256], FP, kind="ExternalInput").ap()
    bu = nc.dram_tensor("bu", [1, NH], FP, kind="ExternalInput").ap()
    bwc = nc.dram_tensor("bwc", [1, 2 * NH], FP, kind="ExternalInput").ap()
    pos = nc.dram_tensor("pos", [8, S], FP, kind="ExternalInput").ap()
    evm = nc.dram_tensor("evm", [P, 2 * DH], FP, kind="ExternalInput").ap()
    out = nc.dram_tensor("out", [S, NH * DH], FP, kind="ExternalOutput").ap()
    us_dram = nc.dram_tensor("us_scratch", [NH, S], FP, kind="Internal").ap()

    with tile.TileContext(nc) as tc:
        with tc.tile_pool(name="const", bufs=1) as pconst, \
             tc.tile_pool(name="persist", bufs=1) as pper:
            # constants
            ones_row = pconst.tile([1, S], FP)
            nc.vector.memset(ones_row[:], 1.0)
            ones_col = pconst.tile([P, 1], FP)
            nc.vector.memset(ones_col[:], 1.0)
            ident = pconst.tile([P, P], FP)
            make_identity(nc, ident[:])
            evm_sb = pconst.tile([P, 2 * DH], FP)
            nc.sync.dma_start(evm_sb[:], evm)
            posL = pconst.tile([4, S], FP)
            nc.sync.dma_start(posL[:], pos[0:4, :])
            posR = pconst.tile([4, S], FP)
            nc.sync.dma_start(posR[:], pos[4:8, :])
            wq_sb = pconst.tile([P, EC, 256], FP)
            nc.sync.dma_start(wq_sb[:], wq.rearrange("(c p) n -> p c n", p=P))
            wk_sb = pconst.tile([P, EC, 256], FP)
            nc.sync.dma_start(wk_sb[:], wk.rearrange("(c p) n -> p c n", p=P))
            wv_sb = pconst.tile([P, EC, 256], FP)
            nc.sync.dma_start(wv_sb[:], wv.rearrange("(c p) n -> p c n", p=P))
            wu_sb = pconst.tile([P, EC, NH], FP)
            nc.sync.dma_start(wu_sb[:], wu.rearrange("(c p) n -> p c n", p=P))
            wwc_sb = pconst.tile([P, EC, 2 * NH], FP)
            nc.sync.dma_start(wwc_sb[:], wwc.rearrange("(c p) n -> p c n", p=P))
            bq_sb = pconst.tile([1, 256], FP)
            nc.sync.dma_start(bq_sb[:], bq)
            bk_sb = pconst.tile([1, 256], FP)
            nc.sync.dma_start(bk_sb[:], bk)
            bv_sb = pconst.tile([1, 256], FP)
            nc.sync.dma_start(bv_sb[:], bv)
            bu_sb = pconst.tile([1, NH], FP)
            nc.sync.dma_start(bu_sb[:], bu)
            bwc_sb = pconst.tile([1, 2 * NH], FP)
            nc.sync.dma_start(bwc_sb[:], bwc)

            # persistent on-chip tensors
            qp_sb = pper.tile([P, 2, S], FP)    # [pair] packed q^T (2 heads/pair)
            kp_sb = pper.tile([P, 2, S], FP)
            us_sb = pper.tile([NH, S], FP)      # u rows (partitions 0..3)
            us_row = pper.tile([1, NH, S], FP)  # u rows per head, on partition 0
            wc_sb = pper.tile([P, R, 2 * NH], FP)  # per r-chunk: w cols 0..3, c1 cols 4..7
            v_all = pper.tile([P, R, NH, DH + 2], FP)  # v + two ones cols
            urep = pper.tile([P, NH, S], FP)    # u broadcast down partitions
            d_all = pper.tile([P, R, S], FP)    # dist tiles (symmetric)

            # ---- Stage A: Q-side projections ----
            with tc.tile_pool(name="psA", bufs=1, space="PSUM") as psA, \
                 tc.tile_pool(name="pinA", bufs=3) as pinA:
                qp_ps = [psA.tile([P, S], FP, tag=f"qp{i}", name=f"qp_ps{i}") for i in range(2)]
                u_ps = psA.tile([NH, S], FP, tag="u")
                for c in range(EC):
                    qTc = pinA.tile([P, S], FP, tag="qin")
                    nc.sync.dma_start(qTc[:], qT[c * P:(c + 1) * P, :])
                    for pair in range(2):
                        for nh in range(2):
                            nc.tensor.matmul(
                                qp_ps[pair][:, bass.ts(nh, 512)],
                                lhsT=wq_sb[:, c, bass.ts(pair, 128)],
                                rhs=qTc[:, bass.ts(nh, 512)],
                                start=(c == 0), stop=False)
                    for nh in range(2):
                        nc.tensor.matmul(
                            u_ps[:, bass.ts(nh, 512)],
                            lhsT=wu_sb[:, c, :],
                            rhs=qTc[:, bass.ts(nh, 512)],
                            start=(c == 0), stop=False)
                # bias via K=1 matmul against ones row
                for pair in range(2):
                    for nh in range(2):
                        nc.tensor.matmul(
                            qp_ps[pair][:, bass.ts(nh, 512)],
                            lhsT=bq_sb[:, bass.ts(pair, 128)],
                            rhs=ones_row[:, bass.ts(nh, 512)],
                            start=False, stop=True)
                for nh in range(2):
                    nc.tensor.matmul(
                        u_ps[:, bass.ts(nh, 512)],
                        lhsT=bu_sb[:],
                        rhs=ones_row[:, bass.ts(nh, 512)],
                        start=False, stop=True)
                for pair in range(2):
                    nc.scalar.copy(qp_sb[:, pair, :], qp_ps[pair][:])
                nc.scalar.copy(us_sb[:], u_ps[:])
                nc.sync.dma_start(us_dram[:, :], us_sb[:])
                nc.sync.dma_start(
                    us_row[0:1, :, :],
                    us_dram.rearrange("h s -> (h s)")[None, :]
                    .rearrange("one (h s) -> one h s", h=NH))

            # ---- Stage B: K-side projections + w/c1 columns ----
            with tc.tile_pool(name="psB", bufs=1, space="PSUM") as psB, \
                 tc.tile_pool(name="psB2", bufs=2, space="PSUM") as psB2, \
                 tc.tile_pool(name="pinB", bufs=4) as pinB:
                kp_ps = [psB.tile([P, S], FP, tag=f"kp{i}", name=f"kp_ps{i}") for i in range(2)]
                kts = []
                for c in range(EC):
                    kTc = pinB.tile([P, S], FP, tag="kin", name=f"kt{c}")
                    nc.sync.dma_start(kTc[:], kT[c * P:(c + 1) * P, :])
                    kts.append(kTc)
                for pair in range(2):
                    for nh in range(2):
                        for c in range(EC):
                            nc.tensor.matmul(
                                kp_ps[pair][:, bass.ts(nh, 512)],
                                lhsT=wk_sb[:, c, bass.ts(pair, 128)],
                                rhs=kts[c][:, bass.ts(nh, 512)],
                                start=(c == 0), stop=False)
                        nc.tensor.matmul(
                            kp_ps[pair][:, bass.ts(nh, 512)],
                            lhsT=bk_sb[:, bass.ts(pair, 128)],
                            rhs=ones_row[:, bass.ts(nh, 512)],
                            start=False, stop=True)
                for pair in range(2):
                    nc.scalar.copy(kp_sb[:, pair, :], kp_ps[pair][:])
                for t in range(R):
                    wcp = psB2.tile([P, 2 * NH], FP, tag="wc")
                    for c in range(EC):
                        nc.tensor.matmul(
                            wcp[:], lhsT=kts[c][:, bass.ts(t, P)],
                            rhs=wwc_sb[:, c, :],
                            start=(c == 0), stop=False)
                    nc.tensor.matmul(
                        wcp[:], lhsT=ones_row[:, 0:P], rhs=bwc_sb[:],
                        start=False, stop=True)
                    nc.scalar.copy(wc_sb[:, t, :], wcp[:])

            # ---- Stage C: V projection ----
            with tc.tile_pool(name="psC", bufs=2, space="PSUM") as psC, \
                 tc.tile_pool(name="pinC", bufs=4) as pinC:
                vts = []
                for c in range(EC):
                    vTc = pinC.tile([P, S], FP, tag="vin", name=f"vt{c}")
                    nc.sync.dma_start(vTc[:], vT[c * P:(c + 1) * P, :])
                    vts.append(vTc)
                for t in range(R):
                    vp = psC.tile([P, 256], FP, tag="v")
                    for c in range(EC):
                        nc.tensor.matmul(
                            vp[:], lhsT=vts[c][:, bass.ts(t, P)],
                            rhs=wv_sb[:, c, :],
                            start=(c == 0), stop=False)
                    nc.tensor.matmul(
                        vp[:], lhsT=ones_row[:, 0:P], rhs=bv_sb[:],
                        start=False, stop=True)
                    nc.scalar.copy(
                        v_all[:, t, :, 0:DH],
                        vp[:].rearrange("p (h d) -> p h d", h=NH))
                nc.vector.memset(v_all[:, :, :, DH:DH + 2], 1.0)

            # ---- u_rep: broadcast u rows down partitions ----
            with tc.tile_pool(name="psU", bufs=2, space="PSUM") as psU:
                for h in range(NH):
                    ur_ps = psU.tile([P, S], FP, tag="ur")
                    for nh in range(2):
                        nc.tensor.matmul(
                            ur_ps[:, bass.ts(nh, 512)],
                            lhsT=ones_row[:, 0:P],
                            rhs=us_row[0:1, h, bass.ts(nh, 512)],
                            start=True, stop=True)
                    nc.scalar.copy(urep[:, h, :], ur_ps[:])

            # ---- dist tiles ----
            with tc.tile_pool(name="psD", bufs=2, space="PSUM") as psD:
                for t in range(R):
                    sq_ps = psD.tile([P, S], FP, tag="sq")
                    for nh in range(2):
                        nc.tensor.matmul(
                            sq_ps[:, bass.ts(nh, 512)],
                            lhsT=posL[:, bass.ts(t, P)],
                            rhs=posR[:, bass.ts(nh, 512)],
                            start=True, stop=True)
                    nc.scalar.activation(
                        d_all[:, t, :], sq_ps[:],
                        mybir.ActivationFunctionType.Sqrt)

            # ---- main attention loop ----
            with tc.tile_pool(name="psS", bufs=2, space="PSUM") as psS, \
                 tc.tile_pool(name="psO", bufs=1, space="PSUM") as psO, \
                 tc.tile_pool(name="work", bufs=2) as pwork, \
                 tc.tile_pool(name="fin", bufs=2) as pfin:
                for h in range(NH):
                    pair, off = h // 2, (h % 2) * DH
                    oacc = psO.tile([DH + 2, 2 * S], FP, tag="oacc")
                    for t in range(R):
                        s1 = psS.tile([P, S], FP, tag="s1")
                        for nh in range(2):
                            nc.tensor.matmul(
                                s1[:, bass.ts(nh, 512)],
                                lhsT=kp_sb[off:off + DH, pair, bass.ts(t, P)],
                                rhs=qp_sb[off:off + DH, pair, bass.ts(nh, 512)],
                                start=True, stop=True)
                        uw = pwork.tile([P, S], FP, tag="uw")
                        nc.vector.tensor_scalar_add(
                            uw[:], urep[:, h, :], wc_sb[:, t, h:h + 1])
                        z = pwork.tile([P, S], FP, tag="z")
                        nc.vector.tensor_tensor(
                            z[:], d_all[:, t, :], uw[:], mybir.AluOpType.mult)
                        a = pwork.tile([P, S], FP, tag="a")
                        nc.vector.tensor_tensor(
                            a[:], z[:], s1[:], mybir.AluOpType.add)
                        e = pwork.tile([P, S], FP, tag="e")
                        nc.scalar.activation(
                            e[:], a[:], mybir.ActivationFunctionType.Exp,
                            bias=wc_sb[:, t, NH + h:NH + h + 1], scale=1.0)
                        ed = pwork.tile([P, S], FP, tag="ed")
                        nc.vector.tensor_tensor(
                            ed[:], e[:], d_all[:, t, :], mybir.AluOpType.mult)
                        for nh in range(2):
                            nc.tensor.matmul(
                                oacc[:, bass.ts(nh, 512)],
                                lhsT=v_all[:, t, h, :],
                                rhs=e[:, bass.ts(nh, 512)],
                                start=(t == 0), stop=(t == R - 1))
                            nc.tensor.matmul(
                                oacc[:, bass.ds(S + nh * 512, 512)],
                                lhsT=v_all[:, t, h, :],
                                rhs=ed[:, bass.ts(nh, 512)],
                                start=(t == 0), stop=(t == R - 1))
                    osb = pfin.tile([DH + 1, S], FP, tag="osb")
                    nc.scalar.copy(osb[:], oacc[0:DH + 1, 0:S])
                    t0sb = pfin.tile([DH + 1, S], FP, tag="t0sb")
                    nc.scalar.copy(t0sb[DH:DH + 1, :], oacc[DH:DH + 1, S:2 * S])
                    for c in range(R):
                        tp = psS.tile([P, DH + 2], FP, tag="s1")
                        nc.tensor.matmul(
                            tp[:, 0:DH + 1], lhsT=osb[:, bass.ts(c, P)],
                            rhs=ident[0:DH + 1, 0:DH + 1],
                            is_transpose=True, start=True, stop=False)
                        nc.tensor.matmul(
                            tp[:, DH + 1:DH + 2],
                            lhsT=t0sb[DH:DH + 1, bass.ts(c, P)],
                            rhs=ident[DH:DH + 1, DH:DH + 1],
                            is_transpose=True, start=False, stop=True)
                        ot = pfin.tile([P, DH + 2], FP, tag="ot")
                        nc.scalar.copy(ot[:], tp[:])
                        rec = pfin.tile([P, 1], FP, tag="rec")
                        nc.vector.reciprocal(rec[:], ot[:, DH:DH + 1])
                        t0r = pfin.tile([P, 1], FP, tag="t0r")
                        nc.vector.tensor_tensor(
                            t0r[:], ot[:, DH + 1:DH + 2], rec[:],
                            mybir.AluOpType.mult)
                        tmp = pfin.tile([P, DH], FP, tag="tmp")
                        nc.vector.scalar_tensor_tensor(
                            tmp[:], evm_sb[:, 0:DH], t0r[:], evm_sb[:, DH:2 * DH],
                            mybir.AluOpType.mult, mybir.AluOpType.add)
                        oput = pfin.tile([P, DH], FP, tag="oput")
                        nc.vector.scalar_tensor_tensor(
                            oput[:], ot[:, 0:DH], rec[:], tmp[:],
                            mybir.AluOpType.mult, mybir.AluOpType.add)
                        nc.sync.dma_start(
                            out[c * P:(c + 1) * P, h * DH:(h + 1) * DH], oput[:])
    nc.finalize()
    _BUILD_CACHE["nc"] = nc
    return nc


def make_core_inputs(inputs, core):
    """Host-side shard prep for one core. inputs: full problem inputs (numpy)."""
    b, hg = core // 2, core % 2
    rows = slice(hg * 256, (hg + 1) * 256)
    f32 = np.float32
    query = np.asarray(inputs["query"], f32)
    key_in = np.asarray(inputs["key_in"], f32)
    value = np.asarray(inputs["value"], f32)
    tp = np.asarray(inputs["tile_positions"], f32)
    Wq = np.asarray(inputs["Wq"], f32); bq = np.asarray(inputs["bq"], f32)
    Wk = np.asarray(inputs["Wk"], f32); bk = np.asarray(inputs["bk"], f32)
    Wv = np.asarray(inputs["Wv"], f32); bv = np.asarray(inputs["bv"], f32)
    Ek = np.asarray(inputs["Ek"], f32)
    Eq = np.asarray(inputs["Eq"], f32)
    Ev = np.asarray(inputs["Ev"], f32)

    alpha = f32(0.25)  # folds 2/sqrt(D) = 2/8 into q
    Wq_h = Wq[rows] * alpha          # [256, 512]
    bq_h = bq[rows] * alpha
    Wk_h = Wk[rows]; bk_h = bk[rows]
    Wv_h = Wv[rows]; bv_h = bv[rows]

    ekd2 = (Ek[0] - Ek[1]) * f32(0.5)   # u = qhat @ ekd2
    eqd8 = (Eq[0] - Eq[1]) * f32(0.125)
    eq18 = Eq[1] * f32(0.125)

    wu = np.stack([Wq_h[h * 64:(h + 1) * 64].T @ ekd2 for h in range(4)], axis=1)
    bu = np.array([[bq_h[h * 64:(h + 1) * 64] @ ekd2 for h in range(4)]], f32)
    wcols = [Wk_h[h * 64:(h + 1) * 64].T @ eqd8 for h in range(4)]
    ccols = [Wk_h[h * 64:(h + 1) * 64].T @ eq18 for h in range(4)]
    wwc = np.stack(wcols + ccols, axis=1)
    bwc = np.array([[bk_h[h * 64:(h + 1) * 64] @ eqd8 for h in range(4)]
                    + [bk_h[h * 64:(h + 1) * 64] @ eq18 for h in range(4)]], f32)

    x = tp[b, :, 0] / f32(MAX_DIST)
    y = tp[b, :, 1] / f32(MAX_DIST)
    sq = x * x + y * y
    ones = np.ones_like(x)
    # +3e-5 guards fp32/PE rounding so sqd stays >= 0 (diag dist ~5.5e-3 vs 0)
    pos = np.stack([-2 * x, -2 * y, sq + f32(3e-5), ones, x, y, ones, sq]).astype(f32)

    dEv = Ev[0] - Ev[1]
    evm = np.concatenate([np.tile(dEv, (P, 1)), np.tile(Ev[1], (P, 1))],
                         axis=1).astype(f32)

    return {
        "qT": np.ascontiguousarray(query[b].T),
        "kT": np.ascontiguousarray(key_in[b].T),
        "vT": np.ascontiguousarray(value[b].T),
        "wq": np.ascontiguousarray(Wq_h.T),
        "wk": np.ascontiguousarray(Wk_h.T),
        "wv": np.ascontiguousarray(Wv_h.T),
        "wu": np.ascontiguousarray(wu),
        "wwc": np.ascontiguousarray(wwc),
        "bq": np.ascontiguousarray(bq_h[None, :]),
        "bk": np.ascontiguousarray(bk_h[None, :]),
        "bv": np.ascontiguousarray(bv_h[None, :]),
        "bu": bu,
        "bwc": bwc,
        "pos": np.ascontiguousarray(pos),
        "evm": evm,
    }


def kernel(**inputs):
    from concourse.bass_utils import run_bass_kernel_spmd
    nc = build_bass()
    in_maps = [make_core_inputs(inputs, core) for core in range(8)]
    res = run_bass_kernel_spmd(nc, in_maps, core_ids=list(range(8)))
    B = 4
    full = np.zeros((B, S, 2 * 256), np.float32)
    for core in range(8):
        b, hg = core // 2, core % 2
        full[b, :, hg * 256:(hg + 1) * 256] = res.results[core]["out"]
    return full
